# revision 24
# baseline (speedup 1.0000x reference)
"""Trainium2 Bass kernel for nn_Attention_5463198400554.

Reference computation (per batch b of 8):
    q    = Wq @ x[b]                      # (N, C) contraction over x's first axis
    attn = scale * q @ x[b].T             # (N, N) contraction over x's second axis
    m    = rowmax(attn)                   # (N, 1)
    v    = colmean(x[b])                  # (1, C)  (mean over tokens)
    out[b][i][j] = v[i] * m[j]            # outer product, (C, N) == (N, C)

Strategy: pure data-parallel over batch — 8 batches on 8 NeuronCores, no
collectives. Key algebraic move: attn = scale * Wq @ G with G = x @ x.T
symmetric, so q is never computed; only G's upper block-triangle is built
by matmul (40 of 64 [128,512] chunks) and the strictly-lower 128-blocks
are mirrored by PE transposes. All transposes (x, Wq, G-lower) run as
regular bf16 matmuls against an identity (out = block.T @ I), which
pipelines at the N=128 matmul rate instead of the slow transpose mode.

Pipeline (single TileContext; Tile owns all semaphores):
  1. x streams in (f32), casts split across ACT/DVE, and is transposed
     tile-by-tile into xT — no resident natural copy of x.
  2. G upper chunks (psum accum over c-blocks) are emitted interleaved
     with the Wq stage (quarter-row staging + transposes into WqT) and
     with the G-lower mirrors as their dependencies complete, so the PE
     stream stays dense; chunk order follows x-group availability.
     v (column sums of x) reduces on ACT via activation accum_out.
  3. attn row-blocks: psum[n-block, m] = sum_i WqT[i, n-block].T @ G[i, m];
     rowmax straight out of PSUM on VectorE. Per-n-block column-wise
     epilogue (DVE 32x32 stream-transpose of m + DRAM broadcast bounce,
     one fused scalar_tensor_tensor, one 1MB output DMA) pipelines with
     the remaining attn matmuls.

The walrus build here caps sync waits at 1 per instruction (2 for
EventSemaphore); _legalize_wait_counts splits Tile's over-capacity waits
onto injected same-engine EventSemaphore carriers post-scheduling.
"""

from contextlib import ExitStack

import numpy as np

import concourse.bass as bass
import concourse.tile as tile
from concourse import mybir
from concourse.bass_utils import run_bass_kernel_spmd
from concourse.masks import make_identity


def _legalize_wait_counts(nc: bass.Bass) -> None:
    """Split over-capacity sync waits onto injected EventSemaphore carriers.

    This walrus build rejects instructions carrying more sync waits than the
    ISA struct holds ("Too many sync wait commands"): 1 wait for ordinary
    instructions, 2 for EventSemaphore. Tile's wait assignment emits more
    (e.g. WAR + RAW on one DMA, or the kernel-tail Drain waiting on every
    DMA queue). Moving excess waits to same-engine EventSemaphore carriers
    immediately before the instruction preserves ordering: the engine blocks
    until those semaphores reach their thresholds, then issues the original
    instruction with the remaining wait.
    """
    counter = [0]
    for blk in nc.m.functions[0].blocks:
        new_insts = []
        changed = False
        for ins in blk.instructions:
            si = ins.sync_info
            waits = list(si.on_wait) if si is not None else []
            cap = 2 if isinstance(ins, mybir.InstEventSemaphore) else 1
            if len(waits) > cap:
                changed = True
                excess, keep = waits[:-cap], waits[-cap:]
                for s in range(0, len(excess), 2):
                    counter[0] += 1
                    ev = mybir.InstEventSemaphore(
                        name=f"waitsplit-{counter[0]}", ins=[], outs=[]
                    )
                    ev.engine = ins.engine
                    ev.sync_info = mybir.SyncInfo(
                        on_wait=excess[s : s + 2], on_update=[]
                    )
                    new_insts.append(ev)
                ins.sync_info = mybir.SyncInfo(
                    on_wait=keep, on_update=list(si.on_update)
                )
            new_insts.append(ins)
        if changed:
            blk.instructions = new_insts

MARKS = []  # (tag, next-inst-id) snapshots for offline cost attribution


def _mark(nc, tag):
    MARKS.append((tag, nc.next_id()))


B = 8
N = 2048  # tokens == channels == dim
P = 128  # partitions
NB = N // P  # 16 blocks of 128
OC = 512  # matmul moving-operand chunk (one PSUM bank of f32)
NOC = N // OC  # 4 chunks
NUM_HEADS = 8
SCALE = (N // NUM_HEADS) ** -0.5  # 1/16
OUT_CONST = SCALE / N  # folds attn scale and the v-mean divisor

F32 = mybir.dt.float32
BF16 = mybir.dt.bfloat16


def build_graph(reps: int = 1) -> bass.Bass:
    nc = bass.Bass(trn_type="TRN2", target_bir_lowering=False, debug=False)
    x_ext = nc.dram_tensor("x", [N, N], F32, kind="ExternalInput").ap()
    wq_ext = nc.dram_tensor("wq", [N, N], F32, kind="ExternalInput").ap()
    out_ext = nc.dram_tensor("out", [N, N], F32, kind="ExternalOutput").ap()

    with tile.TileContext(nc) as tc, ExitStack() as octx:
        consts = octx.enter_context(tc.tile_pool(name="consts", bufs=1))
        ident_bf = consts.tile([P, P], BF16, name="ident_bf")
        make_identity(nc, ident_bf)
        ident_f32 = consts.tile([P, P], F32, name="ident_f32")
        make_identity(nc, ident_f32)
        ones_f32 = consts.tile([1, P], F32, name="ones_f32")
        nc.vector.memset(ones_f32[:], 1.0)
        for rep in range(reps):
            _emit_body(
                nc, tc, x_ext, wq_ext, out_ext, ident_bf, ident_f32, ones_f32, rep
            )

    _legalize_wait_counts(nc)
    return nc


def _emit_body(nc, tc, x_ext, wq_ext, out_ext, ident_bf, ident_f32, ones_f32, rep):
    """v3: attn = scale * Wq @ G with G = x @ x.T (symmetric).

    Emission order puts G's matmuls ahead of the Wq stage so the PE fills
    with G work while Wq streams in through small quarter-row staging tiles
    (the only SBUF left once xt + wqt + G are resident). G's strictly-lower
    128-blocks are mirrored from the upper chunks by PE transposes. The
    epilogue is column-wise and pipelines with the attn phase.
    """
    R = f"r{rep}_"
    with ExitStack() as ctx:
        stats = ctx.enter_context(tc.tile_pool(name=R + "stats", bufs=1))
        dram = ctx.enter_context(tc.tile_pool(name=R + "dram", bufs=16, space="DRAM"))

        v_all = stats.tile([P, NB], F32, name=R + "v_all")  # column sums of x
        # per-(nb, mc) partial row maxes of attn
        m4_all = stats.tile([P, NB, NOC], F32, name=R + "m4_all")

        wqt_pool = ctx.enter_context(
            tc.tile_pool(name=R + "wqt", bufs=1, side="right")
        )
        wqt = wqt_pool.tile([P, NB, N], BF16, name=R + "wqt")  # WqT[i, n]
        g = None

        # attn chunk PSUM: allocated up front (own 2 banks) so interleaved
        # attn chunks never wait on G-phase PSUM WAR chains
        psb1_pool = ctx.enter_context(
            tc.tile_pool(name=R + "psB1", bufs=2, space="PSUM")
        )

        with tc.tile_pool(name=R + "xt", bufs=1) as xt_pool:
            xt = xt_pool.tile([P, NB, N], BF16, name=R + "xt")  # xT[c, m]

            # ---- load x, cast, transpose into xt ----
            # psX (transposes) and psG (G accumulation) coexist so G chunks
            # can start filling PE gaps while later x-groups still stream in.
            pctx = ExitStack()
            psx_pool = pctx.enter_context(
                tc.tile_pool(name=R + "psX", bufs=3, space="PSUM")
            )
            psg_pool = pctx.enter_context(
                tc.tile_pool(name=R + "psG", bufs=3, space="PSUM")
            )
            g0ctx = ExitStack()
            g0_pool = g0ctx.enter_context(tc.tile_pool(name=R + "g0", bufs=1))
            # staging for the 4x4-block G square of x-group 0 (g proper is
            # not yet allocated during the load phase); copied into g after
            # the x staging pools close.
            g0sq = g0_pool.tile([P, 4, OC], BF16, name=R + "g0sq")
            with (
                tc.tile_pool(name=R + "xs", bufs=6) as xs_pool,
                tc.tile_pool(name=R + "xb", bufs=6) as xb_pool,
            ):
                H2 = N // 2
                for ig in range(4):  # groups of 4 x row-blocks
                    xbs = []
                    for k in range(4):
                        i = ig * 4 + k
                        xs = xs_pool.tile([P, N], F32, tag="xs", name=f"{R}xs{i}")
                        xb = xb_pool.tile([P, N], BF16, tag="xb", name=f"{R}xb{i}")
                        # half-row DMA + cast granularity for pipeline start;
                        # casts mostly on idle GPSIMD, 1-in-4 on ACT
                        for h in range(2):
                            sl = slice(h * H2, (h + 1) * H2)
                            _mark(nc, "x_dma")
                            nc.sync.dma_start(
                                xs[:, sl], x_ext[i * P : (i + 1) * P, sl]
                            )
                            _mark(nc, "x_cast")
                            if (2 * i + h) % 4 == 3:
                                nc.scalar.copy(xb[:, sl], xs[:, sl])
                            else:
                                nc.gpsimd.tensor_copy(xb[:, sl], xs[:, sl])
                        xbs.append(xb)
                        if ig == 0:
                            # group 0 is the pipeline ramp: per-block
                            # transposes + block-pair G units start the PE
                            # as soon as each single block lands, instead of
                            # waiting for the whole 4-block group.
                            for s in range(NB):
                                ptb = psx_pool.tile(
                                    [P, P], F32, tag="pt", name=f"{R}ptb{i}_{s}"
                                )
                                _mark(nc, "xT_mm")
                                nc.tensor.matmul(
                                    ptb[:],
                                    xb[:, s * P : (s + 1) * P],
                                    ident_bf[:],
                                    start=True,
                                    stop=True,
                                )
                                _mark(nc, "xT_evac")
                                if s % 4 == 3:
                                    nc.scalar.copy(
                                        xt[:, s, i * P : (i + 1) * P], ptb[:]
                                    )
                                else:
                                    nc.vector.tensor_copy(
                                        xt[:, s, i * P : (i + 1) * P], ptb[:]
                                    )
                            for a in range(k + 1):
                                pgp = psg_pool.tile(
                                    [P, P], F32, tag="pg", name=f"{R}pgp{a}_{k}"
                                )
                                _mark(nc, "g_mm")
                                for cb in range(NB):
                                    nc.tensor.matmul(
                                        pgp[:],
                                        xt[:, cb, a * P : (a + 1) * P],
                                        xt[:, cb, k * P : (k + 1) * P],
                                        start=(cb == 0),
                                        stop=(cb == NB - 1),
                                    )
                                _mark(nc, "g_evac")
                                nc.vector.tensor_copy(
                                    g0sq[:, a, k * P : (k + 1) * P], pgp[:]
                                )
                                if a < k:
                                    plp = psg_pool.tile(
                                        [P, P], F32, tag="pg",
                                        name=f"{R}plp{k}_{a}",
                                    )
                                    _mark(nc, "low_mm")
                                    nc.tensor.matmul(
                                        plp[:],
                                        g0sq[:, a, k * P : (k + 1) * P],
                                        ident_bf[:],
                                        start=True,
                                        stop=True,
                                    )
                                    _mark(nc, "low_evac")
                                    nc.vector.tensor_copy(
                                        g0sq[:, k, a * P : (a + 1) * P], plp[:]
                                    )
                    if ig == 0:
                        continue
                    for s in range(NB):
                        pt = psx_pool.tile(
                            [P, OC], F32, tag="pt", name=f"{R}pt{ig}_{s}"
                        )
                        _mark(nc, "xT_mm")
                        for k in range(4):
                            nc.tensor.matmul(
                                pt[:, k * P : (k + 1) * P],
                                xbs[k][:, s * P : (s + 1) * P],
                                ident_bf[:],
                                start=True,
                                stop=True,
                            )
                        # 1-in-4 evacs go to ACT to level the DVE chain
                        _mark(nc, "xT_evac")
                        if s % 4 == 3:
                            nc.scalar.copy(
                                xt[:, s, ig * OC : (ig + 1) * OC], pt[:]
                            )
                        else:
                            nc.vector.tensor_copy(
                                xt[:, s, ig * OC : (ig + 1) * OC], pt[:]
                            )

            # ---- G = x @ x.T upper chunks; Wq stage emitted after so the
            #      PE prefers G matmuls while Wq DMA streams ----
            g_pool = ctx.enter_context(
                tc.tile_pool(name=R + "g", bufs=1, side="right")
            )
            g = g_pool.tile([P, NB, N], BF16, name=R + "g")  # G[n, m]
            for a in range(4):
                _mark(nc, "g0_copy")
                nc.gpsimd.tensor_copy(g[:, a, 0:OC], g0sq[:, a, :])
            g0ctx.close()
            with (
                tc.tile_pool(name=R + "wqs", bufs=2) as wqs_pool,
                tc.tile_pool(name=R + "wqb", bufs=3) as wqb_pool,
            ):
                QW = OC  # 512-column quarters

                def emit_g_chunk(a, bc):
                    # diagonal chunk starts at the diagonal block; the skipped
                    # sub-diagonal blocks are mirrored from column a instead
                    off = (a % 4) * P if bc == a // 4 else 0
                    pg = psg_pool.tile([P, OC], F32, tag="pg", name=f"{R}pg{a}_{bc}")
                    _mark(nc, "g_mm")
                    for cb in range(NB):
                        nc.tensor.matmul(
                            pg[:, off:OC],
                            xt[:, cb, a * P : (a + 1) * P],
                            xt[:, cb, bc * OC + off : (bc + 1) * OC],
                            start=(cb == 0),
                            stop=(cb == NB - 1),
                        )
                    _mark(nc, "g_evac")
                    nc.vector.tensor_copy(
                        g[:, a, bc * OC + off : (bc + 1) * OC], pg[:, off:OC]
                    )

                def emit_wq_unit(ig, q, unit):
                    # two strided 2-row-block DMAs on the idle SP queue;
                    # casts on idle GPSIMD; evacs alternate DVE/ACT
                    wb2s = []
                    for h in range(2):
                        o0 = ig * 4 + 2 * h
                        ws2 = wqs_pool.tile(
                            [P, 2, QW], F32, tag="ws", name=f"{R}ws{o0}_{q}"
                        )
                        _mark(nc, "wq_dma")
                        nc.sync.dma_start(
                            ws2[:],
                            wq_ext[
                                o0 * P : (o0 + 2) * P, q * QW : (q + 1) * QW
                            ].rearrange("(b p) c -> p b c", p=P),
                        )
                        wb2 = wqb_pool.tile(
                            [P, 2, QW], BF16, tag="wb", name=f"{R}wb{o0}_{q}"
                        )
                        _mark(nc, "wq_cast")
                        nc.gpsimd.tensor_copy(wb2[:], ws2[:])
                        wb2s.append(wb2)
                    for si in range(4):  # i-block s = 4q + si
                        s = 4 * q + si
                        pw = psx_pool.tile(
                            [P, OC], F32, tag="pt", name=f"{R}pw{ig}_{s}"
                        )
                        _mark(nc, "wqT_mm")
                        for k in range(4):
                            nc.tensor.matmul(
                                pw[:, k * P : (k + 1) * P],
                                wb2s[k // 2][:, k % 2, si * P : (si + 1) * P],
                                ident_bf[:],
                                start=True,
                                stop=True,
                            )
                        _mark(nc, "wqT_evac")
                        if (unit * 4 + si) % 2 == 0:
                            nc.vector.tensor_copy(
                                wqt[:, s, ig * OC : (ig + 1) * OC], pw[:]
                            )
                        else:
                            nc.scalar.copy(
                                wqt[:, s, ig * OC : (ig + 1) * OC], pw[:]
                            )

                def emit_g_low(a, bg, w):
                    pl = psg_pool.tile(
                        [P, OC], F32, tag="pg", name=f"{R}pl{a}_{bg}"
                    )
                    _mark(nc, "low_mm")
                    for k in range(w):
                        b = bg * 4 + k
                        nc.tensor.matmul(
                            pl[:, k * P : (k + 1) * P],
                            g[:, b, a * P : (a + 1) * P],
                            ident_bf[:],
                            start=True,
                            stop=True,
                        )
                    _mark(nc, "low_evac")
                    nc.vector.tensor_copy(
                        g[:, a, bg * OC : bg * OC + w * P], pl[:, 0 : w * P]
                    )

                # ordered so chunk (a, bc) is emitted once x-groups
                # max(a//4, bc) have landed -> G starts after group 0.
                # The (a<4, bc=0) square was already built block-pair-wise
                # during the load ramp (g0sq).
                g_chunks = sorted(
                    (
                        (a, bc)
                        for a in range(NB)
                        for bc in range(a // 4, NOC)
                        if not (a < 4 and bc == 0)
                    ),
                    key=lambda t: (max(t[0] // 4, t[1]), t[1], t[0]),
                )
                wq_units = [(ig, q) for ig in range(4) for q in range(4)]
                # lower-mirror group (a, bg, w) covers blocks b in
                # [4bg, 4bg+w); depends on upper chunks (b, a//4).
                # a<4 partial mirrors were handled in the g0sq square.
                low_pending = [
                    (a, bg, 4) for a in range(NB) for bg in range(a // 4)
                ]
                low_pending += [
                    (a, a // 4, a % 4) for a in range(4, NB) if a % 4 > 0
                ]
                done_chunks = {(a, 0) for a in range(4)}

                # readiness tracking for interleaved attn chunks: G column
                # chunk mc is fully resident once every upper chunk with
                # bc==mc and every mirror group with bg==mc has been emitted;
                # wqt columns for nb need all 4 units of ig = nb//4.
                col_remaining = [0] * NOC
                for _a, _bc in g_chunks:
                    col_remaining[_bc] += 1
                for _a, _bg, _w in low_pending:
                    col_remaining[_bg] += 1
                wq_done_q = [0] * 4
                attn_plan = [(mc, nb) for mc in range(NOC) for nb in range(NB)]
                attn_emitted = set()
                ai = 0

                def emit_attn_chunk(nb, mc, pool):
                    pb = pool.tile(
                        [P, OC], F32, tag="pb", name=f"{R}pb{nb}_{mc}"
                    )
                    _mark(nc, "attn_mm")
                    for ib in range(NB):
                        nc.tensor.matmul(
                            pb[:],
                            wqt[:, ib, nb * P : (nb + 1) * P],
                            g[:, ib, mc * OC : (mc + 1) * OC],
                            start=(ib == 0),
                            stop=(ib == NB - 1),
                        )
                    _mark(nc, "rowmax")
                    nc.vector.reduce_max(
                        out=m4_all[:, nb, mc : mc + 1],
                        in_=pb[:],
                        axis=mybir.AxisListType.X,
                    )
                    attn_emitted.add((nb, mc))

                def try_attn(budget):
                    nonlocal ai
                    while ai < len(attn_plan) and budget > 0:
                        mc, nb = attn_plan[ai]
                        if col_remaining[mc] > 0 or wq_done_q[nb // 4] < 4:
                            return
                        emit_attn_chunk(nb, mc, psb1_pool)
                        ai += 1
                        budget -= 1

                def flush_low():
                    nonlocal low_pending
                    rest = []
                    for a, bg, w in low_pending:
                        deps = {(4 * bg + k, a // 4) for k in range(w)}
                        if deps <= done_chunks:
                            emit_g_low(a, bg, w)
                            col_remaining[bg] -= 1
                        else:
                            rest.append((a, bg, w))
                    low_pending = rest

                # v: column sums of x == row sums of xT, scratch-free on DVE
                # (reduce_sum over the free axis), spread through the second
                # half of the wq-unit stream so it never bunches at phase end.
                def emit_v(s):
                    _mark(nc, "v")
                    nc.vector.reduce_sum(
                        out=v_all[:, s : s + 1],
                        in_=xt[:, s, :],
                        axis=mybir.AxisListType.X,
                    )

                v_next = 0
                gi = 0
                for u, (ig, q) in enumerate(wq_units):
                    for _ in range(2 if u < 4 else 1):  # front-load wq units
                        if gi < len(g_chunks):
                            emit_g_chunk(*g_chunks[gi])
                            done_chunks.add(g_chunks[gi])
                            col_remaining[g_chunks[gi][1]] -= 1
                            gi += 1
                    flush_low()
                    emit_wq_unit(ig, q, u)
                    wq_done_q[ig] += 1
                    if u >= 8:
                        for _ in range(2):
                            if v_next < NB:
                                emit_v(v_next)
                                v_next += 1
                    try_attn(1)
                while gi < len(g_chunks):
                    emit_g_chunk(*g_chunks[gi])
                    done_chunks.add(g_chunks[gi])
                    col_remaining[g_chunks[gi][1]] -= 1
                    gi += 1
                    flush_low()
                    try_attn(2)
                assert not low_pending
                while v_next < NB:
                    emit_v(v_next)
                    v_next += 1

        pctx.close()

        # ---- remaining attn chunks, rowmax combine, column-wise epilogue ----
        # mc-outer: each (nb, mc) 512-col chunk accumulates into a single
        # PSUM bank with a partial rowmax per chunk; short kernel tail.
        with (
            tc.tile_pool(name=R + "psB", bufs=4, space="PSUM") as psb_pool,
            tc.tile_pool(name=R + "psE", bufs=1, space="PSUM") as pse_pool,
            tc.tile_pool(name=R + "epi", bufs=3) as epi_pool,
            tc.tile_pool(name=R + "ot", bufs=4) as ot_pool,
        ):

            def emit_epilogue_bounce(nb, mt_in):
                # m column -> row strips via DVE 32x32 stream transpose:
                # mt[32b, c] = m[32b + c]; 4-descriptor DMA to a DRAM row,
                # then partition-broadcast load back.
                mt = epi_pool.tile([P, 32], F32, tag="mt", name=f"{R}mt{nb}")
                _mark(nc, "epi")
                nc.vector.transpose(mt[:], mt_in[:])
                md = dram.tile([1, P], F32, tag="md", name=f"{R}md{nb}")
                strips = bass.AP(
                    tensor=mt.tensor,
                    offset=mt.offset,
                    ap=[[32 * mt.ap[0][0], 4], [1, 32]],
                )
                nc.sync.dma_start(md[0, :].rearrange("(a b) -> a b", a=4), strips)
                m_bc = epi_pool.tile([P, P], F32, tag="mbc", name=f"{R}mb{nb}")
                nc.sync.dma_start(
                    m_bc[:],
                    bass.AP(tensor=md.tensor, offset=md.offset, ap=[[0, P], [1, P]]),
                )
                return m_bc

            def emit_epilogue_pe(nb, mt_in):
                # PE path (short latency, used for the tail blocks): transpose
                # the m column to a PSUM row, bounce through SBUF, then a K=1
                # ones-matmul replicates it across all 128 partitions.
                _mark(nc, "epi")
                pmr = pse_pool.tile([1, P], F32, tag="pmr", name=f"{R}pmr{nb}")
                nc.tensor.matmul(
                    pmr[:], mt_in[:, 0:1], ident_f32[:], start=True, stop=True
                )
                smr = epi_pool.tile([1, P], F32, tag="smr", name=f"{R}smr{nb}")
                nc.vector.tensor_copy(smr[:], pmr[:])
                m_bc = pse_pool.tile([P, P], F32, tag="pbc", name=f"{R}pbc{nb}")
                nc.tensor.matmul(
                    m_bc[:], ones_f32[:], smr[:], start=True, stop=True
                )
                return m_bc

            def emit_store(nb, m_bc, ib0, ib1, ot):
                # out rows [ib0*128, ib1*128) of column block nb
                m_in = bass.AP(
                    tensor=m_bc.tensor,
                    offset=m_bc.offset,
                    ap=[m_bc.ap[0], [0, ib1 - ib0], [1, P]],
                )
                v_in = bass.AP(
                    tensor=v_all.tensor,
                    offset=v_all.offset + ib0 * v_all.ap[1][0],
                    ap=[v_all.ap[0], [v_all.ap[1][0], ib1 - ib0], [0, P]],
                )
                _mark(nc, "stt")
                nc.vector.scalar_tensor_tensor(
                    out=ot[:, ib0:ib1, :],
                    in0=m_in,
                    scalar=OUT_CONST,
                    in1=v_in,
                    op0=mybir.AluOpType.mult,
                    op1=mybir.AluOpType.mult,
                )
                _mark(nc, "out_dma")
                nc.sync.dma_start(
                    out_ext[
                        ib0 * P : ib1 * P, nb * P : (nb + 1) * P
                    ].rearrange("(ib p) j -> p ib j", p=P),
                    ot[:, ib0:ib1, :],
                )

            for nb in range(NB):
                for mc in range(NOC):
                    if (nb, mc) not in attn_emitted:
                        emit_attn_chunk(nb, mc, psb_pool)
                mt_in = epi_pool.tile([P, 32], F32, tag="mti", name=f"{R}mti{nb}")
                _mark(nc, "rowmax")
                nc.vector.reduce_max(
                    out=mt_in[:, 0:1],
                    in_=m4_all[:, nb, 0:NOC],
                    axis=mybir.AxisListType.X,
                )
                ot = ot_pool.tile([P, NB, P], F32, tag="ot", name=f"{R}ot{nb}")
                if nb < NB - 2:
                    m_bc = emit_epilogue_bounce(nb, mt_in)
                    emit_store(nb, m_bc, 0, NB, ot)
                else:
                    # tail blocks: low-latency PE broadcast + split store
                    m_bc = emit_epilogue_pe(nb, mt_in)
                    emit_store(nb, m_bc, 0, NB // 2, ot)
                    emit_store(nb, m_bc, NB // 2, NB, ot)
                _mark(nc, "other")


_NC_CACHE = None


def _get_graph() -> bass.Bass:
    global _NC_CACHE
    if _NC_CACHE is None:
        _NC_CACHE = build_graph()
    return _NC_CACHE


def kernel(x=None, Wq=None, H=None, W=None, **_ignored) -> np.ndarray:
    """Full-input entry point: x (8, 2048, 2048) f32, Wq (2048, 2048) f32.

    Shards batch elements across the 8 NeuronCores (data parallel), runs the
    Bass kernel SPMD, and stacks the per-core outputs back to (8, 2048, 2048).
    H and W are unused by the computation (the reference ignores them).
    """
    x = np.ascontiguousarray(np.asarray(x, dtype=np.float32))
    wq = np.ascontiguousarray(np.asarray(Wq, dtype=np.float32))
    assert x.shape == (B, N, N) and wq.shape == (N, N)

    nc = _get_graph()
    in_maps = [{"x": x[c], "wq": wq} for c in range(B)]
    res = run_bass_kernel_spmd(nc, in_maps, core_ids=list(range(B)))
    return np.stack([res.results[c]["out"] for c in range(B)], axis=0)


if __name__ == "__main__":
    rng = np.random.default_rng(0)
    x = rng.standard_normal((B, N, N), dtype=np.float32)
    wq = (rng.standard_normal((N, N), dtype=np.float32) * 0.02).astype(np.float32)
    out = kernel(x=x, Wq=wq, H=64, W=32)
    print("out shape:", out.shape, out.dtype)

